# revision 51
# baseline (speedup 1.0000x reference)
"""GCN (3x GCNConv + global max pool + FC + log_softmax) on 8 Trainium2 NeuronCores.

Strategy:
  - 1D partition of nodes: core c owns rows [12500c, 12500(c+1)).
  - Per conv layer l: each core computes hs = dinv * (a @ W_l) for its slice
    (PE matmul, bf16). The per-core slice is split into 4 sub-chunks at tile
    boundaries (25/25/25/23 tiles); each sub-chunk is AllGather'd into a
    pair-shared DRAM table as soon as its dense tiles finish, so collectives
    overlap the remaining dense work and the next layer's gathers.
  - Aggregation: per-edge gather of table rows via gpsimd dma_gather (int16
    indices within each 25600-row chunk), then segment-sum via PE matmuls
    with per-128-edge one-hot selection matrices built on DVE (is_equal of
    dst-in-tile index vs an iota row), accumulated in PSUM per 128-dst tile.
    The bias enters as a rank-1 matmul (u=sqrt(deg) outer b) seeding PSUM;
    out = relu(dinv * psum) is a single scalar-engine activation.
  - dense(l+1) for tile t is fused right after agg(l) tile t (PE transpose
    feeds the next matmul), so layers pipeline without phase barriers.
  - Pooling (segment max over graphs), tiny FC and log_softmax run on host
    from the returned conv3 node features (0.01% of FLOPs).
"""

import sys

sys.path.insert(0, "/opt/trn_rl_repo")

import numpy as np
import ml_dtypes

import concourse.bass as bass
import concourse.bacc as bacc
import concourse.tile as tile
from concourse import mybir
from concourse.bass_utils import run_bass_kernel_spmd
from concourse.masks import make_identity

P = 128
N_NODES = 100000
N_EDGES = 1600000
N_GRAPHS = 64
N_CORES = 8
NODES_PER_CORE = N_NODES // N_CORES          # 12500
NTILES = (NODES_PER_CORE + P - 1) // P       # 98 (last tile 84 rows)
LAST_ROWS = NODES_PER_CORE - (NTILES - 1) * P  # 84
NCHUNK = 4
SUBTILES = [25, 25, 25, 23]                  # tiles per sub-chunk
SUBOFF_T = [0, 25, 50, 75]
SUBROWS = [3200, 3200, 3200, 2900]           # rows per core per sub-chunk
SUBOFF_R = [0, 3200, 6400, 9600]
CHUNK_ROWS = [r * N_CORES for r in SUBROWS]  # 25600 x3, 23200
WT = 8                                       # tiles per wave
F0 = 512
FW = 128                                     # table width (all convs padded to 128)
MAX_CALL_BLOCKS = 47                         # ~6K idxs/call; single_packet=False required >1024
NQUEUES = 4
DMA_SCRATCH = 16384
dt = mybir.dt
BF = ml_dtypes.bfloat16


def _rows(t):
    return LAST_ROWS if t == NTILES - 1 else P


def _host_prep(edge_index):
    """Build the shared (cross-core) aggregation schedule + per-core index data."""
    src = np.concatenate([edge_index[0], np.arange(N_NODES, dtype=np.int64)]).astype(np.int64)
    dst = np.concatenate([edge_index[1], np.arange(N_NODES, dtype=np.int64)]).astype(np.int64)
    deg = np.bincount(dst, minlength=N_NODES).astype(np.float32)
    dinv = (1.0 / np.sqrt(deg)).astype(np.float32)
    drecip = np.sqrt(deg).astype(np.float32)  # 1/dinv, for the rank-1 bias matmul

    waves = [list(range(w, min(w + WT, NTILES))) for w in range(0, NTILES, WT)]

    # chunk id + index-in-chunk for every source node (sub-chunk row split)
    s_core = src // NODES_PER_CORE
    s_rl = src % NODES_PER_CORE
    ch_of = np.digitize(s_rl, SUBOFF_R[1:])
    suboff = np.asarray(SUBOFF_R, np.int64)[ch_of]
    subrows = np.asarray(SUBROWS, np.int64)[ch_of]
    s_inchunk = s_core * subrows + (s_rl - suboff)

    core_of = dst // NODES_PER_CORE
    per_core = []
    cnts = np.zeros((N_CORES, NTILES, NCHUNK), np.int64)
    for c in range(N_CORES):
        m = core_of == c
        s, d = s_inchunk[m], dst[m]
        chm = ch_of[m]
        dl = d - c * NODES_PER_CORE
        t = dl // P
        key = t * NCHUNK + chm
        o = np.argsort(key, kind="stable")
        s, dl, key = s[o], dl[o], key[o]
        cnt = np.bincount(key, minlength=NTILES * NCHUNK).reshape(NTILES, NCHUNK)
        cnts[c] = cnt
        per_core.append((s, dl, cnt))

    blocks = np.maximum((cnts.max(axis=0) + P - 1) // P, 1)  # [NTILES, NCHUNK] shared

    S_ch = [int(blocks[:, ch].sum() * P) for ch in range(NCHUNK)]
    NB_total = int(blocks.sum())

    idx16 = [np.zeros((N_CORES, 128, S_ch[ch] // 16), np.int16) for ch in range(NCHUNK)]
    didx = np.full((N_CORES, 128, NB_total), -1.0, np.float32)

    chunk_start = [dict() for _ in range(NCHUNK)]
    for ch in range(NCHUNK):
        pos = 0
        for w, wtiles in enumerate(waves):
            for t in wtiles:
                chunk_start[ch][t] = pos
                pos += int(blocks[t, ch]) * P
    g_start = {}
    g = 0
    for w, wtiles in enumerate(waves):
        for t in wtiles:
            for ch in range(NCHUNK):
                g_start[(t, ch)] = g
                g += int(blocks[t, ch])
    assert g == NB_total

    for c in range(N_CORES):
        s, dl, cnt = per_core[c]
        ends = np.cumsum(cnt.reshape(-1))
        starts = ends - cnt.reshape(-1)
        idx_slots = [np.zeros(S_ch[ch], np.int16) for ch in range(NCHUNK)]
        didx_slots = np.full(NB_total * P, -1.0, np.float32)
        for t in range(NTILES):
            for ch in range(NCHUNK):
                k = t * NCHUNK + ch
                n = cnt[t, ch]
                if n == 0:
                    continue
                e0, e1 = starts[k], ends[k]
                ss = s[e0:e1]
                dd = dl[e0:e1] - t * P
                o2 = np.argsort(ss, kind="stable")  # ascending srcs: HBM locality
                ss, dd = ss[o2], dd[o2]
                cs = chunk_start[ch][t]
                idx_slots[ch][cs:cs + n] = ss.astype(np.int16)
                gs = g_start[(t, ch)] * P
                didx_slots[gs:gs + n] = dd.astype(np.float32)
        for ch in range(NCHUNK):
            w16 = idx_slots[ch].reshape(-1, 16).T  # [16, S/16]
            idx16[ch][c] = np.tile(w16, (8, 1))
        didx[c] = didx_slots.reshape(-1, P).T

    meta = {
        "waves": waves,
        "blocks": blocks,
        "S_ch": S_ch,
        "NB_total": NB_total,
        "chunk_start": chunk_start,
        "g_start": g_start,
    }
    return dinv, drecip, idx16, didx, meta


def _build_program(meta):
    waves = meta["waves"]
    blocks = meta["blocks"]
    S_ch = meta["S_ch"]
    NB_total = meta["NB_total"]
    chunk_start = meta["chunk_start"]
    g_start = meta["g_start"]

    nc = bacc.Bacc(
        "TRN2", target_bir_lowering=False, debug=False, num_devices=N_CORES,
        num_swdge_queues=NQUEUES, dynamic_dma_scratch_size=DMA_SCRATCH,
    )

    xT_io = nc.dram_tensor("xT", [F0, NODES_PER_CORE], dt.bfloat16, kind="ExternalInput").ap()
    dinv_io = nc.dram_tensor("dinvT", [P, NTILES], dt.float32, kind="ExternalInput").ap()
    w1_io = nc.dram_tensor("W1sb", [P, F0], dt.bfloat16, kind="ExternalInput").ap()
    w2_io = nc.dram_tensor("W2pad", [P, P], dt.bfloat16, kind="ExternalInput").ap()
    w3_io = nc.dram_tensor("W3pad", [P, P], dt.bfloat16, kind="ExternalInput").ap()
    b_io = nc.dram_tensor("bvecs", [4, P], dt.bfloat16, kind="ExternalInput").ap()
    uT_io = nc.dram_tensor("uT", [1, NODES_PER_CORE], dt.bfloat16, kind="ExternalInput").ap()
    iota_io = nc.dram_tensor("iota", [P, P], dt.bfloat16, kind="ExternalInput").ap()
    idx_ios = [
        nc.dram_tensor(f"idx{ch}", [P, S_ch[ch] // 16], dt.int16, kind="ExternalInput").ap()
        for ch in range(NCHUNK)
    ]
    didx_io = nc.dram_tensor("didx", [P, NB_total], dt.bfloat16, kind="ExternalInput").ap()
    out_io = nc.dram_tensor("out3", [NODES_PER_CORE, 32], dt.float32, kind="ExternalOutput").ap()

    with tile.TileContext(nc) as tc:
        with (
            tc.tile_pool(name="const", bufs=1) as constp,
            tc.tile_pool(name="idxw", bufs=10) as idxp,
            tc.tile_pool(name="msgs", bufs=9) as msgp,
            tc.tile_pool(name="uTw", bufs=2) as uTp,
            tc.tile_pool(name="sel", bufs=12) as selp,
            tc.tile_pool(name="work", bufs=3) as workp,
            tc.tile_pool(name="hs", bufs=3) as hsp,
            tc.tile_pool(name="aT", bufs=3) as aTp,
            tc.tile_pool(name="psum_d", bufs=2, space="PSUM") as psdp,
            tc.tile_pool(name="psum_a", bufs=4, space="PSUM") as psap,
            tc.tile_pool(name="psum_t", bufs=2, space="PSUM") as pstp,
            tc.tile_pool(name="dram", bufs=1, space="DRAM") as dramp,
        ):
            # ---- constants ----
            iota_t = constp.tile([P, P], dt.bfloat16)
            nc.sync.dma_start(iota_t[:], iota_io[:])
            dinv_sb = constp.tile([P, NTILES], dt.float32)
            nc.sync.dma_start(dinv_sb[:], dinv_io[:])
            w1_sb = constp.tile([P, F0], dt.bfloat16)
            nc.sync.dma_start(w1_sb[:], w1_io[:])
            w2_sb = constp.tile([P, P], dt.bfloat16)
            nc.sync.dma_start(w2_sb[:], w2_io[:])
            w3_sb = constp.tile([P, P], dt.bfloat16)
            nc.sync.dma_start(w3_sb[:], w3_io[:])
            b_sbs = []
            for l in range(3):
                b_l = constp.tile([1, P], dt.bfloat16, name=f"b_sb{l}")
                nc.sync.dma_start(b_l[:], b_io[l:l + 1, :])
                b_sbs.append(b_l)
            didx_sb = constp.tile([P, NB_total], dt.bfloat16)
            nc.sync.dma_start(didx_sb[:], didx_io[:])
            ident = constp.tile([P, P], dt.float32)
            make_identity(nc, ident[:])

            import os as _os
            _shared = "Shared" if _os.environ.get("GCN_SHARED_TBL", "1") == "1" else "Local"
            tbls = [
                [
                    dramp.tile([CHUNK_ROWS[k], FW], dt.bfloat16,
                               tag=f"tbl{l}_{k}", name=f"tbl{l}_{k}", addr_space=_shared)
                    for k in range(NCHUNK)
                ]
                for l in range(3)
            ]
            bounces = [
                dramp.tile([NODES_PER_CORE, FW], dt.bfloat16, tag=f"bnc{l}", name=f"bnc{l}")
                for l in range(3)
            ]

            def sub_allgather(l, k):
                nc.gpsimd.collective_compute(
                    "AllGather", mybir.AluOpType.bypass,
                    replica_groups=[list(range(N_CORES))],
                    ins=[bounces[l][SUBOFF_R[k]:SUBOFF_R[k] + SUBROWS[k], :].opt()],
                    outs=[tbls[l][k].opt()],
                )

            boundary_of = {SUBOFF_T[k] + SUBTILES[k] - 1: k for k in range(NCHUNK)}

            def dense_tile(l, t, lhsT_full, w_sb, col0=0):
                """hs_t = dinv * (a_t @ W_l) -> bounce[l]."""
                r = _rows(t)
                c0 = t * P
                ps = psdp.tile([r, P], dt.float32, space="PSUM", tag="pd")
                if l == 0:
                    nk = F0 // P
                    for k in range(nk):
                        nc.tensor.matmul(
                            out=ps[:], lhsT=lhsT_full[:, k, col0:col0 + r],
                            rhs=w_sb[:, k * P:(k + 1) * P],
                            start=(k == 0), stop=(k == nk - 1),
                        )
                else:
                    nc.tensor.matmul(
                        out=ps[:], lhsT=lhsT_full[:, :r], rhs=w_sb[:],
                        start=True, stop=True,
                    )
                hs = hsp.tile([r, P], dt.bfloat16, tag="hs")
                nc.scalar.activation(
                    hs[:], ps[:], mybir.ActivationFunctionType.Copy,
                    scale=dinv_sb[:r, t:t + 1],
                )
                # scalar-engine DGE: keeps the sync queue free for idx loads
                nc.scalar.dma_start(bounces[l][c0:c0 + r, :], hs[:])
                if t in boundary_of:
                    sub_allgather(l, boundary_of[t])

            # ---- layer-0 dense: wave-sized bulk xT loads, then matmul per tile ----
            xT_r = xT_io.rearrange("(k p) n -> p k n", p=P)
            with tc.tile_pool(name="xT", bufs=2) as xTp:
                for w, wtiles in enumerate(waves):
                    c0w = wtiles[0] * P
                    cols = sum(_rows(t) for t in wtiles)
                    xw = xTp.tile([P, F0 // P, WT * P], dt.bfloat16, tag="xw")
                    nc.sync.dma_start(xw[:, :, :cols], xT_r[:, :, c0w:c0w + cols])
                    for ti, t in enumerate(wtiles):
                        dense_tile(0, t, xw, w1_sb, col0=ti * P)

            def agg_phase(l):
                """agg from tbls[l]; out = relu(dinv*(agg + u x b)); fused dense(l+1)."""
                ncols = P if l < 2 else 32

                def gather_one(w, ch):
                    wtiles = waves[w]
                    nb = int(sum(blocks[t, ch] for t in wtiles))
                    s0 = chunk_start[ch][wtiles[0]]
                    S = nb * P
                    iw = idxp.tile([P, S // 16], dt.int16, tag="idx")
                    nc.sync.dma_start(iw[:], idx_ios[ch][:, s0 // 16:(s0 + S) // 16])
                    mt = msgp.tile([P, nb, FW], dt.bfloat16, tag="msg")
                    q = ch % NQUEUES
                    for b0 in range(0, nb, MAX_CALL_BLOCKS):
                        b1 = min(b0 + MAX_CALL_BLOCKS, nb)
                        Ssub = (b1 - b0) * P
                        nc.gpsimd.dma_gather(
                            out_ap=mt[:, b0:b1, :],
                            in_ap=tbls[l][ch][:],
                            idxs_ap=iw[:, b0 * P // 16:b1 * P // 16],
                            num_idxs=Ssub, num_idxs_reg=Ssub,
                            elem_size=FW, elem_step=FW,
                            single_packet=False,
                            queue_num=q,
                        )
                    return mt

                for w, wtiles in enumerate(waves):
                    msg_tiles = {ch: gather_one(w, ch) for ch in range(NCHUNK)}
                    uTw = uTp.tile([1, WT * P], dt.bfloat16, tag="uTw")
                    c0w = wtiles[0] * P
                    colsw = sum(_rows(t) for t in wtiles)
                    nc.sync.dma_start(uTw[0:1, :colsw], uT_io[0:1, c0w:c0w + colsw])

                    # selection matrices, batches of BB blocks in didx col order
                    gw0 = g_start[(wtiles[0], 0)]
                    gw1 = gw0 + int(sum(blocks[t, ch] for t in wtiles for ch in range(NCHUNK)))
                    BB = 16
                    sel_tiles = {}
                    for q0 in range(gw0, gw1, BB):
                        q1 = min(q0 + BB, gw1)
                        st = selp.tile([P, BB, P], dt.bfloat16, tag="sel")
                        nc.vector.tensor_tensor(
                            out=st[:, :q1 - q0, :],
                            in0=didx_sb[:, q0:q1, None].to_broadcast([P, q1 - q0, P]),
                            in1=iota_t[:, None, :].to_broadcast([P, q1 - q0, P]),
                            op=mybir.AluOpType.is_equal,
                        )
                        for q in range(q0, q1):
                            sel_tiles[q] = (st, q - q0)

                    # per-tile matmul accumulation + post + fused next dense
                    for ti, t in enumerate(wtiles):
                        r = _rows(t)
                        c0 = t * P
                        ps = psap.tile([r, ncols], dt.float32, space="PSUM", tag="pa")
                        # rank-1 bias seed: ps = u[dst] (x) b, so that
                        # dinv * (agg + u (x) b) = dinv*agg + b
                        nc.tensor.matmul(
                            out=ps[:], lhsT=uTw[0:1, ti * P:ti * P + r],
                            rhs=b_sbs[l][0:1, :ncols],
                            start=True, stop=False,
                        )
                        mms = []
                        for ch in range(NCHUNK):
                            coff = int(sum(blocks[tt, ch] for tt in wtiles[:ti]))
                            for b in range(int(blocks[t, ch])):
                                mms.append((g_start[(t, ch)] + b, ch, coff + b))
                        for i, (q, ch, col) in enumerate(mms):
                            st, j = sel_tiles[q]
                            nc.tensor.matmul(
                                out=ps[:], lhsT=st[:, j, :r],
                                rhs=msg_tiles[ch][:, col, :ncols],
                                start=False, stop=(i == len(mms) - 1),
                            )
                        outt = workp.tile([P, ncols], dt.float32, tag="outt")
                        nc.scalar.activation(
                            outt[:r], ps[:], mybir.ActivationFunctionType.Relu,
                            scale=dinv_sb[:r, t:t + 1],
                        )
                        if l < 2:
                            pst = pstp.tile([P, P], dt.float32, space="PSUM", tag="pt")
                            nc.tensor.transpose(out=pst[:, :r], in_=outt[:r], identity=ident[:r, :r])
                            aTt = aTp.tile([P, P], dt.bfloat16, tag="aTt")
                            # scalar engine: keep DVE free for sel builds
                            nc.scalar.activation(
                                aTt[:, :r], pst[:, :r],
                                mybir.ActivationFunctionType.Copy,
                            )
                            dense_tile(l + 1, t, aTt, w2_sb if l == 0 else w3_sb)
                        else:
                            nc.scalar.dma_start(out_io[c0:c0 + r, :], outt[:r])

            for l in range(3):
                agg_phase(l)

    nc.compile()
    return nc


def _pack_inputs(x, dinv, drecip, W1, b1, W2, b2, W3, b3, idx16, didx):
    iota_rep = np.tile(np.arange(P, dtype=np.float32)[None, :], (P, 1)).astype(BF)
    # W1 packed: W1sb[i, 128k+j] = W1[128k+i, j]
    w1sb = np.zeros((P, F0), np.float32)
    for k in range(F0 // P):
        w1sb[:, k * P:(k + 1) * P] = W1[k * P:(k + 1) * P, :]
    w2pad = np.zeros((P, P), np.float32)
    w2pad[:, :64] = W2
    w3pad = np.zeros((P, P), np.float32)
    w3pad[:64, :32] = W3
    bvecs = np.zeros((4, P), np.float32)
    bvecs[0, :128] = b1
    bvecs[1, :64] = b2
    bvecs[2, :32] = b3

    in_maps = []
    for c in range(N_CORES):
        lo = c * NODES_PER_CORE
        xs = x[lo:lo + NODES_PER_CORE].astype(np.float32)
        dvt = np.ones((P, NTILES), np.float32)
        dv = dinv[lo:lo + NODES_PER_CORE]
        for t in range(NTILES):
            r = _rows(t)
            dvt[:r, t] = dv[t * P:t * P + r]
        in_maps.append({
            "xT": np.ascontiguousarray(xs.T).astype(BF),
            "dinvT": dvt,
            "W1sb": w1sb.astype(BF),
            "W2pad": w2pad.astype(BF),
            "W3pad": w3pad.astype(BF),
            "bvecs": bvecs.astype(BF),
            "uT": drecip[None, lo:lo + NODES_PER_CORE].astype(BF),
            "iota": iota_rep,
            **{f"idx{ch}": idx16[ch][c] for ch in range(NCHUNK)},
            "didx": didx[c].astype(BF),
        })
    return in_maps


_TRACE = [False]          # set by test harness to request a profiled run
_LAST_RESULT = [None]     # BassKernelResults of the last run (for profiling)


def kernel(x, edge_index, batch, W1, b1, W2, b2, W3, b3, Wfc, bfc):
    x = np.asarray(x)
    edge_index = np.asarray(edge_index)
    batch = np.asarray(batch)
    W1, b1 = np.asarray(W1), np.asarray(b1)
    W2, b2 = np.asarray(W2), np.asarray(b2)
    W3, b3 = np.asarray(W3), np.asarray(b3)
    Wfc, bfc = np.asarray(Wfc), np.asarray(bfc)

    dinv, drecip, idx16, didx, meta = _host_prep(edge_index.astype(np.int64))
    nc = _build_program(meta)
    in_maps = _pack_inputs(x, dinv, drecip, W1, b1, W2, b2, W3, b3, idx16, didx)
    res = run_bass_kernel_spmd(
        nc, in_maps, core_ids=list(range(N_CORES)), trace=_TRACE[0]
    )
    _LAST_RESULT[0] = res

    h3 = np.concatenate([res.results[c]["out3"][:, :32] for c in range(N_CORES)], axis=0)

    # host epilogue: segment max pool + FC + log_softmax (float64 for stability)
    pooled = np.full((N_GRAPHS, 32), -np.inf, np.float64)
    bnd = np.searchsorted(batch, np.arange(N_GRAPHS + 1))
    for g in range(N_GRAPHS):
        if bnd[g + 1] > bnd[g]:
            pooled[g] = h3[bnd[g]:bnd[g + 1]].max(axis=0)
    logits = pooled @ Wfc.astype(np.float64) + bfc.astype(np.float64)
    m = logits.max(axis=1, keepdims=True)
    lse = m + np.log(np.exp(logits - m).sum(axis=1, keepdims=True))
    return (logits - lse).astype(np.float32)


# revision 56
# speedup vs baseline: 1.0494x; 1.0494x over previous
"""GCN (3x GCNConv + global max pool + FC + log_softmax) on 8 Trainium2 NeuronCores.

Strategy:
  - 1D partition of nodes: core c owns rows [12500c, 12500(c+1)).
  - Per conv layer l: each core computes hs = dinv * (a @ W_l) for its slice
    (PE matmul, bf16). The per-core slice is split into 4 sub-chunks at tile
    boundaries (25/25/25/23 tiles); each sub-chunk is AllGather'd into a
    pair-shared DRAM table as soon as its dense tiles finish, so collectives
    overlap the remaining dense work and the next layer's gathers.
  - Aggregation: per-edge gather of table rows via gpsimd dma_gather (int16
    indices within each 25600-row chunk), then segment-sum via PE matmuls
    with per-128-edge one-hot selection matrices built on DVE (is_equal of
    dst-in-tile index vs an iota row), accumulated in PSUM per 128-dst tile.
    The bias enters as a rank-1 matmul (u=sqrt(deg) outer b) seeding PSUM;
    out = relu(dinv * psum) is a single scalar-engine activation.
  - dense(l+1) for tile t is fused right after agg(l) tile t (PE transpose
    feeds the next matmul), so layers pipeline without phase barriers.
  - Pooling (segment max over graphs), tiny FC and log_softmax run on host
    from the returned conv3 node features (0.01% of FLOPs).
"""

import sys

sys.path.insert(0, "/opt/trn_rl_repo")

import numpy as np
import ml_dtypes

import concourse.bass as bass
import concourse.bacc as bacc
import concourse.tile as tile
from concourse import mybir
from concourse.bass_utils import run_bass_kernel_spmd
from concourse.masks import make_identity

P = 128
N_NODES = 100000
N_EDGES = 1600000
N_GRAPHS = 64
N_CORES = 8
NODES_PER_CORE = N_NODES // N_CORES          # 12500
NTILES = (NODES_PER_CORE + P - 1) // P       # 98 (last tile 84 rows)
LAST_ROWS = NODES_PER_CORE - (NTILES - 1) * P  # 84
NCHUNK = 4
SUBTILES = [25, 25, 25, 23]                  # tiles per sub-chunk
SUBOFF_T = [0, 25, 50, 75]
SUBROWS = [3200, 3200, 3200, 2900]           # rows per core per sub-chunk
SUBOFF_R = [0, 3200, 6400, 9600]
CHUNK_ROWS = [r * N_CORES for r in SUBROWS]  # 25600 x3, 23200
WT = 8                                       # tiles per wave
F0 = 512
FW = 128                                     # table width (all convs padded to 128)
MAX_CALL_BLOCKS = 47                         # ~6K idxs/call; single_packet=False required >1024
NQUEUES = 4
DMA_SCRATCH = 16384
dt = mybir.dt
BF = ml_dtypes.bfloat16


def _rows(t):
    return LAST_ROWS if t == NTILES - 1 else P


def _host_prep(edge_index):
    """Build the shared (cross-core) aggregation schedule + per-core index data."""
    src = np.concatenate([edge_index[0], np.arange(N_NODES, dtype=np.int64)]).astype(np.int64)
    dst = np.concatenate([edge_index[1], np.arange(N_NODES, dtype=np.int64)]).astype(np.int64)
    deg = np.bincount(dst, minlength=N_NODES).astype(np.float32)
    dinv = (1.0 / np.sqrt(deg)).astype(np.float32)
    drecip = np.sqrt(deg).astype(np.float32)  # 1/dinv, for the rank-1 bias matmul

    waves = [list(range(w, min(w + WT, NTILES))) for w in range(0, NTILES, WT)]

    # chunk id + index-in-chunk for every source node (sub-chunk row split)
    s_core = src // NODES_PER_CORE
    s_rl = src % NODES_PER_CORE
    ch_of = np.digitize(s_rl, SUBOFF_R[1:])
    suboff = np.asarray(SUBOFF_R, np.int64)[ch_of]
    subrows = np.asarray(SUBROWS, np.int64)[ch_of]
    s_inchunk = s_core * subrows + (s_rl - suboff)

    core_of = dst // NODES_PER_CORE
    per_core = []
    cnts = np.zeros((N_CORES, NTILES, NCHUNK), np.int64)
    for c in range(N_CORES):
        m = core_of == c
        s, d = s_inchunk[m], dst[m]
        chm = ch_of[m]
        dl = d - c * NODES_PER_CORE
        t = dl // P
        key = t * NCHUNK + chm
        o = np.argsort(key, kind="stable")
        s, dl, key = s[o], dl[o], key[o]
        cnt = np.bincount(key, minlength=NTILES * NCHUNK).reshape(NTILES, NCHUNK)
        cnts[c] = cnt
        per_core.append((s, dl, cnt))

    blocks = np.maximum((cnts.max(axis=0) + P - 1) // P, 1)  # [NTILES, NCHUNK] shared

    S_ch = [int(blocks[:, ch].sum() * P) for ch in range(NCHUNK)]
    NB_total = int(blocks.sum())

    idx16 = [np.zeros((N_CORES, 128, S_ch[ch] // 16), np.int16) for ch in range(NCHUNK)]
    didx = np.full((N_CORES, 128, NB_total), -1.0, np.float32)

    chunk_start = [dict() for _ in range(NCHUNK)]
    for ch in range(NCHUNK):
        pos = 0
        for w, wtiles in enumerate(waves):
            for t in wtiles:
                chunk_start[ch][t] = pos
                pos += int(blocks[t, ch]) * P
    g_start = {}
    g = 0
    for w, wtiles in enumerate(waves):
        for t in wtiles:
            for ch in range(NCHUNK):
                g_start[(t, ch)] = g
                g += int(blocks[t, ch])
    assert g == NB_total

    for c in range(N_CORES):
        s, dl, cnt = per_core[c]
        ends = np.cumsum(cnt.reshape(-1))
        starts = ends - cnt.reshape(-1)
        idx_slots = [np.zeros(S_ch[ch], np.int16) for ch in range(NCHUNK)]
        didx_slots = np.full(NB_total * P, -1.0, np.float32)
        for t in range(NTILES):
            for ch in range(NCHUNK):
                k = t * NCHUNK + ch
                n = cnt[t, ch]
                if n == 0:
                    continue
                e0, e1 = starts[k], ends[k]
                ss = s[e0:e1]
                dd = dl[e0:e1] - t * P
                o2 = np.argsort(ss, kind="stable")  # ascending srcs: HBM locality
                ss, dd = ss[o2], dd[o2]
                cs = chunk_start[ch][t]
                idx_slots[ch][cs:cs + n] = ss.astype(np.int16)
                gs = g_start[(t, ch)] * P
                didx_slots[gs:gs + n] = dd.astype(np.float32)
        for ch in range(NCHUNK):
            w16 = idx_slots[ch].reshape(-1, 16).T  # [16, S/16]
            idx16[ch][c] = np.tile(w16, (8, 1))
        didx[c] = didx_slots.reshape(-1, P).T

    meta = {
        "waves": waves,
        "blocks": blocks,
        "S_ch": S_ch,
        "NB_total": NB_total,
        "chunk_start": chunk_start,
        "g_start": g_start,
    }
    return dinv, drecip, idx16, didx, meta


def _build_program(meta):
    waves = meta["waves"]
    blocks = meta["blocks"]
    S_ch = meta["S_ch"]
    NB_total = meta["NB_total"]
    chunk_start = meta["chunk_start"]
    g_start = meta["g_start"]

    nc = bacc.Bacc(
        "TRN2", target_bir_lowering=False, debug=False, num_devices=N_CORES,
        num_swdge_queues=NQUEUES, dynamic_dma_scratch_size=DMA_SCRATCH,
    )

    xT_io = nc.dram_tensor("xT", [F0, NODES_PER_CORE], dt.bfloat16, kind="ExternalInput").ap()
    dinv_io = nc.dram_tensor("dinvT", [P, NTILES], dt.float32, kind="ExternalInput").ap()
    w1_io = nc.dram_tensor("W1sb", [P, F0], dt.bfloat16, kind="ExternalInput").ap()
    w2_io = nc.dram_tensor("W2pad", [P, P], dt.bfloat16, kind="ExternalInput").ap()
    w3_io = nc.dram_tensor("W3pad", [P, P], dt.bfloat16, kind="ExternalInput").ap()
    b_io = nc.dram_tensor("bvecs", [4, P], dt.bfloat16, kind="ExternalInput").ap()
    uT_io = nc.dram_tensor("uT", [1, NODES_PER_CORE], dt.bfloat16, kind="ExternalInput").ap()
    iota_io = nc.dram_tensor("iota", [P, P], dt.bfloat16, kind="ExternalInput").ap()
    idx_ios = [
        nc.dram_tensor(f"idx{ch}", [P, S_ch[ch] // 16], dt.int16, kind="ExternalInput").ap()
        for ch in range(NCHUNK)
    ]
    didx_io = nc.dram_tensor("didx", [P, NB_total], dt.bfloat16, kind="ExternalInput").ap()
    out_io = nc.dram_tensor("out3", [NODES_PER_CORE, 32], dt.float32, kind="ExternalOutput").ap()

    with tile.TileContext(nc) as tc:
        with (
            tc.tile_pool(name="const", bufs=1) as constp,
            tc.tile_pool(name="idxw", bufs=10) as idxp,
            tc.tile_pool(name="msgs", bufs=9) as msgp,
            tc.tile_pool(name="uTw", bufs=2) as uTp,
            tc.tile_pool(name="sel", bufs=14) as selp,
            tc.tile_pool(name="work", bufs=3) as workp,
            tc.tile_pool(name="hs", bufs=3) as hsp,
            tc.tile_pool(name="aT", bufs=3) as aTp,
            tc.tile_pool(name="psum_d", bufs=2, space="PSUM") as psdp,
            tc.tile_pool(name="psum_a", bufs=4, space="PSUM") as psap,
            tc.tile_pool(name="psum_t", bufs=2, space="PSUM") as pstp,
            tc.tile_pool(name="dram", bufs=1, space="DRAM") as dramp,
        ):
            # ---- constants ----
            iota_t = constp.tile([P, P], dt.bfloat16)
            nc.sync.dma_start(iota_t[:], iota_io[:])
            dinv_sb = constp.tile([P, NTILES], dt.float32)
            nc.sync.dma_start(dinv_sb[:], dinv_io[:])
            w1_sb = constp.tile([P, F0], dt.bfloat16)
            nc.sync.dma_start(w1_sb[:], w1_io[:])
            w2_sb = constp.tile([P, P], dt.bfloat16)
            nc.sync.dma_start(w2_sb[:], w2_io[:])
            w3_sb = constp.tile([P, P], dt.bfloat16)
            nc.sync.dma_start(w3_sb[:], w3_io[:])
            b_sbs = []
            for l in range(3):
                b_l = constp.tile([1, P], dt.bfloat16, name=f"b_sb{l}")
                nc.sync.dma_start(b_l[:], b_io[l:l + 1, :])
                b_sbs.append(b_l)
            didx_sb = constp.tile([P, NB_total], dt.bfloat16)
            nc.sync.dma_start(didx_sb[:], didx_io[:])
            ident = constp.tile([P, P], dt.float32)
            make_identity(nc, ident[:])

            import os as _os
            _shared = "Shared" if _os.environ.get("GCN_SHARED_TBL", "1") == "1" else "Local"
            tbls = [
                [
                    dramp.tile([CHUNK_ROWS[k], FW], dt.bfloat16,
                               tag=f"tbl{l}_{k}", name=f"tbl{l}_{k}", addr_space=_shared)
                    for k in range(NCHUNK)
                ]
                for l in range(3)
            ]
            bounces = [
                dramp.tile([NODES_PER_CORE, FW], dt.bfloat16, tag=f"bnc{l}", name=f"bnc{l}")
                for l in range(3)
            ]

            def sub_allgather(l, k):
                nc.gpsimd.collective_compute(
                    "AllGather", mybir.AluOpType.bypass,
                    replica_groups=[list(range(N_CORES))],
                    ins=[bounces[l][SUBOFF_R[k]:SUBOFF_R[k] + SUBROWS[k], :].opt()],
                    outs=[tbls[l][k].opt()],
                )

            boundary_of = {SUBOFF_T[k] + SUBTILES[k] - 1: k for k in range(NCHUNK)}

            def dense_tile(l, t, lhsT_full, w_sb, col0=0):
                """hs_t = dinv * (a_t @ W_l) -> bounce[l]."""
                r = _rows(t)
                c0 = t * P
                ps = psdp.tile([r, P], dt.float32, space="PSUM", tag="pd")
                if l == 0:
                    nk = F0 // P
                    for k in range(nk):
                        nc.tensor.matmul(
                            out=ps[:], lhsT=lhsT_full[:, k, col0:col0 + r],
                            rhs=w_sb[:, k * P:(k + 1) * P],
                            start=(k == 0), stop=(k == nk - 1),
                        )
                else:
                    nc.tensor.matmul(
                        out=ps[:], lhsT=lhsT_full[:, :r], rhs=w_sb[:],
                        start=True, stop=True,
                    )
                hs = hsp.tile([r, P], dt.bfloat16, tag="hs")
                nc.scalar.activation(
                    hs[:], ps[:], mybir.ActivationFunctionType.Copy,
                    scale=dinv_sb[:r, t:t + 1],
                )
                # scalar-engine DGE: keeps the sync queue free for idx loads
                nc.scalar.dma_start(bounces[l][c0:c0 + r, :], hs[:])
                if t in boundary_of:
                    sub_allgather(l, boundary_of[t])

            # ---- layer-0 dense: wave-sized bulk xT loads, then matmul per tile ----
            xT_r = xT_io.rearrange("(k p) n -> p k n", p=P)
            with tc.tile_pool(name="xT", bufs=2) as xTp:
                for w, wtiles in enumerate(waves):
                    c0w = wtiles[0] * P
                    cols = sum(_rows(t) for t in wtiles)
                    xw = xTp.tile([P, F0 // P, WT * P], dt.bfloat16, tag="xw")
                    nc.sync.dma_start(xw[:, :, :cols], xT_r[:, :, c0w:c0w + cols])
                    for ti, t in enumerate(wtiles):
                        dense_tile(0, t, xw, w1_sb, col0=ti * P)

            def agg_phase(l):
                """agg from tbls[l]; out = relu(dinv*(agg + u x b)); fused dense(l+1)."""
                ncols = P if l < 2 else 32

                def gather_one(w, ch):
                    wtiles = waves[w]
                    nb = int(sum(blocks[t, ch] for t in wtiles))
                    s0 = chunk_start[ch][wtiles[0]]
                    S = nb * P
                    iw = idxp.tile([P, S // 16], dt.int16, tag="idx")
                    nc.sync.dma_start(iw[:], idx_ios[ch][:, s0 // 16:(s0 + S) // 16])
                    mt = msgp.tile([P, nb, FW], dt.bfloat16, tag="msg")
                    q = ch % NQUEUES
                    for b0 in range(0, nb, MAX_CALL_BLOCKS):
                        b1 = min(b0 + MAX_CALL_BLOCKS, nb)
                        Ssub = (b1 - b0) * P
                        nc.gpsimd.dma_gather(
                            out_ap=mt[:, b0:b1, :],
                            in_ap=tbls[l][ch][:],
                            idxs_ap=iw[:, b0 * P // 16:b1 * P // 16],
                            num_idxs=Ssub, num_idxs_reg=Ssub,
                            elem_size=FW, elem_step=FW,
                            single_packet=False,
                            queue_num=q,
                        )
                    return mt

                for w, wtiles in enumerate(waves):
                    msg_tiles = {ch: gather_one(w, ch) for ch in range(NCHUNK)}
                    # per-wave u slice on the scalar DGE: keeps the sync queue
                    # (gather idx loads) clean and frees 25KB/part of const SBUF
                    uTw = uTp.tile([1, WT * P], dt.bfloat16, tag="uTw")
                    c0w = wtiles[0] * P
                    colsw = sum(_rows(t) for t in wtiles)
                    nc.scalar.dma_start(uTw[0:1, :colsw], uT_io[0:1, c0w:c0w + colsw])

                    # selection matrices, batches of BB blocks in didx col order
                    gw0 = g_start[(wtiles[0], 0)]
                    gw1 = gw0 + int(sum(blocks[t, ch] for t in wtiles for ch in range(NCHUNK)))
                    BB = 16
                    sel_tiles = {}
                    for q0 in range(gw0, gw1, BB):
                        q1 = min(q0 + BB, gw1)
                        st = selp.tile([P, BB, P], dt.bfloat16, tag="sel")
                        nc.vector.tensor_tensor(
                            out=st[:, :q1 - q0, :],
                            in0=didx_sb[:, q0:q1, None].to_broadcast([P, q1 - q0, P]),
                            in1=iota_t[:, None, :].to_broadcast([P, q1 - q0, P]),
                            op=mybir.AluOpType.is_equal,
                        )
                        for q in range(q0, q1):
                            sel_tiles[q] = (st, q - q0)

                    # per-tile matmul accumulation + post + fused next dense
                    for ti, t in enumerate(wtiles):
                        r = _rows(t)
                        c0 = t * P
                        ps = psap.tile([r, ncols], dt.float32, space="PSUM", tag="pa")
                        # rank-1 bias seed: ps = u[dst] (x) b, so that
                        # dinv * (agg + u (x) b) = dinv*agg + b
                        nc.tensor.matmul(
                            out=ps[:], lhsT=uTw[0:1, ti * P:ti * P + r],
                            rhs=b_sbs[l][0:1, :ncols],
                            start=True, stop=False,
                        )
                        mms = []
                        for ch in range(NCHUNK):
                            coff = int(sum(blocks[tt, ch] for tt in wtiles[:ti]))
                            for b in range(int(blocks[t, ch])):
                                mms.append((g_start[(t, ch)] + b, ch, coff + b))
                        for i, (q, ch, col) in enumerate(mms):
                            st, j = sel_tiles[q]
                            nc.tensor.matmul(
                                out=ps[:], lhsT=st[:, j, :r],
                                rhs=msg_tiles[ch][:, col, :ncols],
                                start=False, stop=(i == len(mms) - 1),
                            )
                        outt = workp.tile([P, ncols], dt.float32, tag="outt")
                        nc.scalar.activation(
                            outt[:r], ps[:], mybir.ActivationFunctionType.Relu,
                            scale=dinv_sb[:r, t:t + 1],
                        )
                        if l < 2:
                            pst = pstp.tile([P, P], dt.float32, space="PSUM", tag="pt")
                            nc.tensor.transpose(out=pst[:, :r], in_=outt[:r], identity=ident[:r, :r])
                            aTt = aTp.tile([P, P], dt.bfloat16, tag="aTt")
                            # scalar engine: keep DVE free for sel builds
                            nc.scalar.activation(
                                aTt[:, :r], pst[:, :r],
                                mybir.ActivationFunctionType.Copy,
                            )
                            dense_tile(l + 1, t, aTt, w2_sb if l == 0 else w3_sb)
                        else:
                            nc.scalar.dma_start(out_io[c0:c0 + r, :], outt[:r])

            for l in range(3):
                agg_phase(l)

    nc.compile()
    return nc


def _pack_inputs(x, dinv, drecip, W1, b1, W2, b2, W3, b3, idx16, didx):
    iota_rep = np.tile(np.arange(P, dtype=np.float32)[None, :], (P, 1)).astype(BF)
    # W1 packed: W1sb[i, 128k+j] = W1[128k+i, j]
    w1sb = np.zeros((P, F0), np.float32)
    for k in range(F0 // P):
        w1sb[:, k * P:(k + 1) * P] = W1[k * P:(k + 1) * P, :]
    w2pad = np.zeros((P, P), np.float32)
    w2pad[:, :64] = W2
    w3pad = np.zeros((P, P), np.float32)
    w3pad[:64, :32] = W3
    bvecs = np.zeros((4, P), np.float32)
    bvecs[0, :128] = b1
    bvecs[1, :64] = b2
    bvecs[2, :32] = b3

    in_maps = []
    for c in range(N_CORES):
        lo = c * NODES_PER_CORE
        xs = x[lo:lo + NODES_PER_CORE].astype(np.float32)
        dvt = np.ones((P, NTILES), np.float32)
        dv = dinv[lo:lo + NODES_PER_CORE]
        for t in range(NTILES):
            r = _rows(t)
            dvt[:r, t] = dv[t * P:t * P + r]
        in_maps.append({
            "xT": np.ascontiguousarray(xs.T).astype(BF),
            "dinvT": dvt,
            "W1sb": w1sb.astype(BF),
            "W2pad": w2pad.astype(BF),
            "W3pad": w3pad.astype(BF),
            "bvecs": bvecs.astype(BF),
            "uT": drecip[None, lo:lo + NODES_PER_CORE].astype(BF),
            "iota": iota_rep,
            **{f"idx{ch}": idx16[ch][c] for ch in range(NCHUNK)},
            "didx": didx[c].astype(BF),
        })
    return in_maps


_TRACE = [False]          # set by test harness to request a profiled run
_LAST_RESULT = [None]     # BassKernelResults of the last run (for profiling)


def kernel(x, edge_index, batch, W1, b1, W2, b2, W3, b3, Wfc, bfc):
    x = np.asarray(x)
    edge_index = np.asarray(edge_index)
    batch = np.asarray(batch)
    W1, b1 = np.asarray(W1), np.asarray(b1)
    W2, b2 = np.asarray(W2), np.asarray(b2)
    W3, b3 = np.asarray(W3), np.asarray(b3)
    Wfc, bfc = np.asarray(Wfc), np.asarray(bfc)

    dinv, drecip, idx16, didx, meta = _host_prep(edge_index.astype(np.int64))
    nc = _build_program(meta)
    in_maps = _pack_inputs(x, dinv, drecip, W1, b1, W2, b2, W3, b3, idx16, didx)
    res = run_bass_kernel_spmd(
        nc, in_maps, core_ids=list(range(N_CORES)), trace=_TRACE[0]
    )
    _LAST_RESULT[0] = res

    h3 = np.concatenate([res.results[c]["out3"][:, :32] for c in range(N_CORES)], axis=0)

    # host epilogue: segment max pool + FC + log_softmax (float64 for stability)
    pooled = np.full((N_GRAPHS, 32), -np.inf, np.float64)
    bnd = np.searchsorted(batch, np.arange(N_GRAPHS + 1))
    for g in range(N_GRAPHS):
        if bnd[g + 1] > bnd[g]:
            pooled[g] = h3[bnd[g]:bnd[g + 1]].max(axis=0)
    logits = pooled @ Wfc.astype(np.float64) + bfc.astype(np.float64)
    m = logits.max(axis=1, keepdims=True)
    lse = m + np.log(np.exp(logits - m).sum(axis=1, keepdims=True))
    return (logits - lse).astype(np.float32)


# revision 58
# speedup vs baseline: 1.1496x; 1.0955x over previous
"""GCN (3x GCNConv + global max pool + FC + log_softmax) on 8 Trainium2 NeuronCores.

Strategy:
  - 1D partition of nodes: core c owns rows [12500c, 12500(c+1)).
  - Per conv layer l: each core computes hs = dinv * (a @ W_l) for its slice
    (PE matmul, bf16). The per-core slice is split into 4 sub-chunks at tile
    boundaries (25/25/25/23 tiles); each sub-chunk is AllGather'd into a
    pair-shared DRAM table as soon as its dense tiles finish, so collectives
    overlap the remaining dense work and the next layer's gathers.
  - Aggregation: per-edge gather of table rows via gpsimd dma_gather (int16
    indices within each 25600-row chunk), then segment-sum via PE matmuls
    with per-128-edge one-hot selection matrices built on DVE (is_equal of
    dst-in-tile index vs an iota row), accumulated in PSUM per 128-dst tile.
    The bias enters as a rank-1 matmul (u=sqrt(deg) outer b) seeding PSUM;
    out = relu(dinv * psum) is a single scalar-engine activation.
  - dense(l+1) for tile t is fused right after agg(l) tile t (PE transpose
    feeds the next matmul), so layers pipeline without phase barriers.
  - Pooling (segment max over graphs), tiny FC and log_softmax run on host
    from the returned conv3 node features (0.01% of FLOPs).
"""

import sys

sys.path.insert(0, "/opt/trn_rl_repo")

import numpy as np
import ml_dtypes

import concourse.bass as bass
import concourse.bacc as bacc
import concourse.tile as tile
from concourse import mybir
from concourse.bass_utils import run_bass_kernel_spmd
from concourse.masks import make_identity

P = 128
N_NODES = 100000
N_EDGES = 1600000
N_GRAPHS = 64
N_CORES = 8
NODES_PER_CORE = N_NODES // N_CORES          # 12500
NTILES = (NODES_PER_CORE + P - 1) // P       # 98 (last tile 84 rows)
LAST_ROWS = NODES_PER_CORE - (NTILES - 1) * P  # 84
NCHUNK = 4
SUBTILES = [26, 26, 26, 20]                  # tiles per sub-chunk; slightly smaller tail
SUBOFF_T = [0, 26, 52, 78]                   # so the last sub-AllGather (layer gate) is shorter
SUBROWS = [3328, 3328, 3328, 2516]           # rows per core per sub-chunk
SUBOFF_R = [0, 3328, 6656, 9984]
CHUNK_ROWS = [r * N_CORES for r in SUBROWS]  # 25600 x3, 23200
WT = 8                                       # tiles per wave
F0 = 512
FW = 128                                     # table width (all convs padded to 128)
MAX_CALL_BLOCKS = 47                         # ~6K idxs/call; single_packet=False required >1024
NQUEUES = 4
DMA_SCRATCH = 16384
dt = mybir.dt
BF = ml_dtypes.bfloat16


def _rows(t):
    return LAST_ROWS if t == NTILES - 1 else P


def _host_prep(edge_index):
    """Build the shared (cross-core) aggregation schedule + per-core index data."""
    src = np.concatenate([edge_index[0], np.arange(N_NODES, dtype=np.int64)]).astype(np.int64)
    dst = np.concatenate([edge_index[1], np.arange(N_NODES, dtype=np.int64)]).astype(np.int64)
    deg = np.bincount(dst, minlength=N_NODES).astype(np.float32)
    dinv = (1.0 / np.sqrt(deg)).astype(np.float32)
    drecip = np.sqrt(deg).astype(np.float32)  # 1/dinv, for the rank-1 bias matmul

    waves = [list(range(w, min(w + WT, NTILES))) for w in range(0, NTILES, WT)]

    # chunk id + index-in-chunk for every source node (sub-chunk row split)
    s_core = src // NODES_PER_CORE
    s_rl = src % NODES_PER_CORE
    ch_of = np.digitize(s_rl, SUBOFF_R[1:])
    suboff = np.asarray(SUBOFF_R, np.int64)[ch_of]
    subrows = np.asarray(SUBROWS, np.int64)[ch_of]
    s_inchunk = s_core * subrows + (s_rl - suboff)

    core_of = dst // NODES_PER_CORE
    per_core = []
    cnts = np.zeros((N_CORES, NTILES, NCHUNK), np.int64)
    for c in range(N_CORES):
        m = core_of == c
        s, d = s_inchunk[m], dst[m]
        chm = ch_of[m]
        dl = d - c * NODES_PER_CORE
        t = dl // P
        key = t * NCHUNK + chm
        o = np.argsort(key, kind="stable")
        s, dl, key = s[o], dl[o], key[o]
        cnt = np.bincount(key, minlength=NTILES * NCHUNK).reshape(NTILES, NCHUNK)
        cnts[c] = cnt
        per_core.append((s, dl, cnt))

    blocks = np.maximum((cnts.max(axis=0) + P - 1) // P, 1)  # [NTILES, NCHUNK] shared

    S_ch = [int(blocks[:, ch].sum() * P) for ch in range(NCHUNK)]
    NB_total = int(blocks.sum())

    idx16 = [np.zeros((N_CORES, 128, S_ch[ch] // 16), np.int16) for ch in range(NCHUNK)]
    didx = np.full((N_CORES, 128, NB_total), -1.0, np.float32)

    chunk_start = [dict() for _ in range(NCHUNK)]
    for ch in range(NCHUNK):
        pos = 0
        for w, wtiles in enumerate(waves):
            for t in wtiles:
                chunk_start[ch][t] = pos
                pos += int(blocks[t, ch]) * P
    g_start = {}
    g = 0
    for w, wtiles in enumerate(waves):
        for t in wtiles:
            for ch in range(NCHUNK):
                g_start[(t, ch)] = g
                g += int(blocks[t, ch])
    assert g == NB_total

    for c in range(N_CORES):
        s, dl, cnt = per_core[c]
        ends = np.cumsum(cnt.reshape(-1))
        starts = ends - cnt.reshape(-1)
        idx_slots = [np.zeros(S_ch[ch], np.int16) for ch in range(NCHUNK)]
        didx_slots = np.full(NB_total * P, -1.0, np.float32)
        for t in range(NTILES):
            for ch in range(NCHUNK):
                k = t * NCHUNK + ch
                n = cnt[t, ch]
                if n == 0:
                    continue
                e0, e1 = starts[k], ends[k]
                ss = s[e0:e1]
                dd = dl[e0:e1] - t * P
                o2 = np.argsort(ss, kind="stable")  # ascending srcs: HBM locality
                ss, dd = ss[o2], dd[o2]
                cs = chunk_start[ch][t]
                idx_slots[ch][cs:cs + n] = ss.astype(np.int16)
                gs = g_start[(t, ch)] * P
                didx_slots[gs:gs + n] = dd.astype(np.float32)
        for ch in range(NCHUNK):
            w16 = idx_slots[ch].reshape(-1, 16).T  # [16, S/16]
            idx16[ch][c] = np.tile(w16, (8, 1))
        didx[c] = didx_slots.reshape(-1, P).T

    meta = {
        "waves": waves,
        "blocks": blocks,
        "S_ch": S_ch,
        "NB_total": NB_total,
        "chunk_start": chunk_start,
        "g_start": g_start,
    }
    return dinv, drecip, idx16, didx, meta


def _build_program(meta):
    waves = meta["waves"]
    blocks = meta["blocks"]
    S_ch = meta["S_ch"]
    NB_total = meta["NB_total"]
    chunk_start = meta["chunk_start"]
    g_start = meta["g_start"]

    nc = bacc.Bacc(
        "TRN2", target_bir_lowering=False, debug=False, num_devices=N_CORES,
        num_swdge_queues=NQUEUES, dynamic_dma_scratch_size=DMA_SCRATCH,
    )

    xT_io = nc.dram_tensor("xT", [F0, NODES_PER_CORE], dt.bfloat16, kind="ExternalInput").ap()
    dinv_io = nc.dram_tensor("dinvT", [P, NTILES], dt.float32, kind="ExternalInput").ap()
    w1_io = nc.dram_tensor("W1sb", [P, F0], dt.bfloat16, kind="ExternalInput").ap()
    w2_io = nc.dram_tensor("W2pad", [P, P], dt.bfloat16, kind="ExternalInput").ap()
    w3_io = nc.dram_tensor("W3pad", [P, P], dt.bfloat16, kind="ExternalInput").ap()
    b_io = nc.dram_tensor("bvecs", [4, P], dt.bfloat16, kind="ExternalInput").ap()
    uT_io = nc.dram_tensor("uT", [1, NODES_PER_CORE], dt.bfloat16, kind="ExternalInput").ap()
    iota_io = nc.dram_tensor("iota", [P, P], dt.bfloat16, kind="ExternalInput").ap()
    idx_ios = [
        nc.dram_tensor(f"idx{ch}", [P, S_ch[ch] // 16], dt.int16, kind="ExternalInput").ap()
        for ch in range(NCHUNK)
    ]
    didx_io = nc.dram_tensor("didx", [P, NB_total], dt.bfloat16, kind="ExternalInput").ap()
    out_io = nc.dram_tensor("out3", [NODES_PER_CORE, 32], dt.float32, kind="ExternalOutput").ap()

    with tile.TileContext(nc) as tc:
        with (
            tc.tile_pool(name="const", bufs=1) as constp,
            tc.tile_pool(name="idxw", bufs=10) as idxp,
            tc.tile_pool(name="msgs", bufs=9) as msgp,
            tc.tile_pool(name="uTw", bufs=2) as uTp,
            tc.tile_pool(name="sel", bufs=14) as selp,
            tc.tile_pool(name="work", bufs=3) as workp,
            tc.tile_pool(name="hs", bufs=3) as hsp,
            tc.tile_pool(name="aT", bufs=3) as aTp,
            tc.tile_pool(name="psum_d", bufs=2, space="PSUM") as psdp,
            tc.tile_pool(name="psum_a", bufs=4, space="PSUM") as psap,
            tc.tile_pool(name="psum_t", bufs=2, space="PSUM") as pstp,
            tc.tile_pool(name="dram", bufs=1, space="DRAM") as dramp,
        ):
            # ---- constants ----
            iota_t = constp.tile([P, P], dt.bfloat16)
            nc.sync.dma_start(iota_t[:], iota_io[:])
            dinv_sb = constp.tile([P, NTILES], dt.float32)
            nc.sync.dma_start(dinv_sb[:], dinv_io[:])
            w1_sb = constp.tile([P, F0], dt.bfloat16)
            nc.sync.dma_start(w1_sb[:], w1_io[:])
            w2_sb = constp.tile([P, P], dt.bfloat16)
            nc.sync.dma_start(w2_sb[:], w2_io[:])
            w3_sb = constp.tile([P, P], dt.bfloat16)
            nc.sync.dma_start(w3_sb[:], w3_io[:])
            b_sbs = []
            for l in range(3):
                b_l = constp.tile([1, P], dt.bfloat16, name=f"b_sb{l}")
                nc.sync.dma_start(b_l[:], b_io[l:l + 1, :])
                b_sbs.append(b_l)
            didx_sb = constp.tile([P, NB_total], dt.bfloat16)
            nc.sync.dma_start(didx_sb[:], didx_io[:])
            ident = constp.tile([P, P], dt.float32)
            make_identity(nc, ident[:])

            import os as _os
            _shared = "Shared" if _os.environ.get("GCN_SHARED_TBL", "1") == "1" else "Local"
            tbls = [
                [
                    dramp.tile([CHUNK_ROWS[k], FW], dt.bfloat16,
                               tag=f"tbl{l}_{k}", name=f"tbl{l}_{k}", addr_space=_shared)
                    for k in range(NCHUNK)
                ]
                for l in range(3)
            ]
            bounces = [
                dramp.tile([NODES_PER_CORE, FW], dt.bfloat16, tag=f"bnc{l}", name=f"bnc{l}")
                for l in range(3)
            ]

            def sub_allgather(l, k):
                nc.gpsimd.collective_compute(
                    "AllGather", mybir.AluOpType.bypass,
                    replica_groups=[list(range(N_CORES))],
                    ins=[bounces[l][SUBOFF_R[k]:SUBOFF_R[k] + SUBROWS[k], :].opt()],
                    outs=[tbls[l][k].opt()],
                )

            boundary_of = {SUBOFF_T[k] + SUBTILES[k] - 1: k for k in range(NCHUNK)}

            def dense_tile(l, t, lhsT_full, w_sb, col0=0):
                """hs_t = dinv * (a_t @ W_l) -> bounce[l]."""
                r = _rows(t)
                c0 = t * P
                ps = psdp.tile([r, P], dt.float32, space="PSUM", tag="pd")
                if l == 0:
                    nk = F0 // P
                    for k in range(nk):
                        nc.tensor.matmul(
                            out=ps[:], lhsT=lhsT_full[:, k, col0:col0 + r],
                            rhs=w_sb[:, k * P:(k + 1) * P],
                            start=(k == 0), stop=(k == nk - 1),
                        )
                else:
                    nc.tensor.matmul(
                        out=ps[:], lhsT=lhsT_full[:, :r], rhs=w_sb[:],
                        start=True, stop=True,
                    )
                hs = hsp.tile([r, P], dt.bfloat16, tag="hs")
                nc.scalar.activation(
                    hs[:], ps[:], mybir.ActivationFunctionType.Copy,
                    scale=dinv_sb[:r, t:t + 1],
                )
                # scalar-engine DGE: keeps the sync queue free for idx loads
                nc.scalar.dma_start(bounces[l][c0:c0 + r, :], hs[:])
                if t in boundary_of:
                    sub_allgather(l, boundary_of[t])

            # ---- layer-0 dense: wave-sized bulk xT loads, then matmul per tile ----
            xT_r = xT_io.rearrange("(k p) n -> p k n", p=P)
            with tc.tile_pool(name="xT", bufs=2) as xTp:
                for w, wtiles in enumerate(waves):
                    c0w = wtiles[0] * P
                    cols = sum(_rows(t) for t in wtiles)
                    xw = xTp.tile([P, F0 // P, WT * P], dt.bfloat16, tag="xw")
                    nc.sync.dma_start(xw[:, :, :cols], xT_r[:, :, c0w:c0w + cols])
                    for ti, t in enumerate(wtiles):
                        dense_tile(0, t, xw, w1_sb, col0=ti * P)

            def agg_phase(l):
                """agg from tbls[l]; out = relu(dinv*(agg + u x b)); fused dense(l+1)."""
                ncols = P if l < 2 else 32

                def gather_one(w, ch):
                    wtiles = waves[w]
                    nb = int(sum(blocks[t, ch] for t in wtiles))
                    s0 = chunk_start[ch][wtiles[0]]
                    S = nb * P
                    iw = idxp.tile([P, S // 16], dt.int16, tag="idx")
                    nc.sync.dma_start(iw[:], idx_ios[ch][:, s0 // 16:(s0 + S) // 16])
                    mt = msgp.tile([P, nb, FW], dt.bfloat16, tag="msg")
                    q = ch % NQUEUES
                    for b0 in range(0, nb, MAX_CALL_BLOCKS):
                        b1 = min(b0 + MAX_CALL_BLOCKS, nb)
                        Ssub = (b1 - b0) * P
                        nc.gpsimd.dma_gather(
                            out_ap=mt[:, b0:b1, :],
                            in_ap=tbls[l][ch][:],
                            idxs_ap=iw[:, b0 * P // 16:b1 * P // 16],
                            num_idxs=Ssub, num_idxs_reg=Ssub,
                            elem_size=FW, elem_step=FW,
                            single_packet=False,
                            queue_num=q,
                        )
                    return mt

                for w, wtiles in enumerate(waves):
                    msg_tiles = {ch: gather_one(w, ch) for ch in range(NCHUNK)}
                    # per-wave u slice on the scalar DGE: keeps the sync queue
                    # (gather idx loads) clean and frees 25KB/part of const SBUF
                    uTw = uTp.tile([1, WT * P], dt.bfloat16, tag="uTw")
                    c0w = wtiles[0] * P
                    colsw = sum(_rows(t) for t in wtiles)
                    nc.scalar.dma_start(uTw[0:1, :colsw], uT_io[0:1, c0w:c0w + colsw])

                    # selection matrices, batches of BB blocks in didx col order
                    gw0 = g_start[(wtiles[0], 0)]
                    gw1 = gw0 + int(sum(blocks[t, ch] for t in wtiles for ch in range(NCHUNK)))
                    BB = 16
                    sel_tiles = {}
                    for q0 in range(gw0, gw1, BB):
                        q1 = min(q0 + BB, gw1)
                        st = selp.tile([P, BB, P], dt.bfloat16, tag="sel")
                        nc.vector.tensor_tensor(
                            out=st[:, :q1 - q0, :],
                            in0=didx_sb[:, q0:q1, None].to_broadcast([P, q1 - q0, P]),
                            in1=iota_t[:, None, :].to_broadcast([P, q1 - q0, P]),
                            op=mybir.AluOpType.is_equal,
                        )
                        for q in range(q0, q1):
                            sel_tiles[q] = (st, q - q0)

                    # per-tile matmul accumulation + post + fused next dense
                    for ti, t in enumerate(wtiles):
                        r = _rows(t)
                        c0 = t * P
                        ps = psap.tile([r, ncols], dt.float32, space="PSUM", tag="pa")
                        # rank-1 bias seed: ps = u[dst] (x) b, so that
                        # dinv * (agg + u (x) b) = dinv*agg + b
                        nc.tensor.matmul(
                            out=ps[:], lhsT=uTw[0:1, ti * P:ti * P + r],
                            rhs=b_sbs[l][0:1, :ncols],
                            start=True, stop=False,
                        )
                        mms = []
                        for ch in range(NCHUNK):
                            coff = int(sum(blocks[tt, ch] for tt in wtiles[:ti]))
                            for b in range(int(blocks[t, ch])):
                                mms.append((g_start[(t, ch)] + b, ch, coff + b))
                        for i, (q, ch, col) in enumerate(mms):
                            st, j = sel_tiles[q]
                            nc.tensor.matmul(
                                out=ps[:], lhsT=st[:, j, :r],
                                rhs=msg_tiles[ch][:, col, :ncols],
                                start=False, stop=(i == len(mms) - 1),
                            )
                        outt = workp.tile([P, ncols], dt.float32, tag="outt")
                        nc.scalar.activation(
                            outt[:r], ps[:], mybir.ActivationFunctionType.Relu,
                            scale=dinv_sb[:r, t:t + 1],
                        )
                        if l < 2:
                            pst = pstp.tile([P, P], dt.float32, space="PSUM", tag="pt")
                            nc.tensor.transpose(out=pst[:, :r], in_=outt[:r], identity=ident[:r, :r])
                            aTt = aTp.tile([P, P], dt.bfloat16, tag="aTt")
                            # scalar engine: keep DVE free for sel builds
                            nc.scalar.activation(
                                aTt[:, :r], pst[:, :r],
                                mybir.ActivationFunctionType.Copy,
                            )
                            dense_tile(l + 1, t, aTt, w2_sb if l == 0 else w3_sb)
                        else:
                            nc.scalar.dma_start(out_io[c0:c0 + r, :], outt[:r])

            for l in range(3):
                agg_phase(l)

    nc.compile()
    return nc


def _pack_inputs(x, dinv, drecip, W1, b1, W2, b2, W3, b3, idx16, didx):
    iota_rep = np.tile(np.arange(P, dtype=np.float32)[None, :], (P, 1)).astype(BF)
    # W1 packed: W1sb[i, 128k+j] = W1[128k+i, j]
    w1sb = np.zeros((P, F0), np.float32)
    for k in range(F0 // P):
        w1sb[:, k * P:(k + 1) * P] = W1[k * P:(k + 1) * P, :]
    w2pad = np.zeros((P, P), np.float32)
    w2pad[:, :64] = W2
    w3pad = np.zeros((P, P), np.float32)
    w3pad[:64, :32] = W3
    bvecs = np.zeros((4, P), np.float32)
    bvecs[0, :128] = b1
    bvecs[1, :64] = b2
    bvecs[2, :32] = b3

    in_maps = []
    for c in range(N_CORES):
        lo = c * NODES_PER_CORE
        xs = x[lo:lo + NODES_PER_CORE].astype(np.float32)
        dvt = np.ones((P, NTILES), np.float32)
        dv = dinv[lo:lo + NODES_PER_CORE]
        for t in range(NTILES):
            r = _rows(t)
            dvt[:r, t] = dv[t * P:t * P + r]
        in_maps.append({
            "xT": np.ascontiguousarray(xs.T).astype(BF),
            "dinvT": dvt,
            "W1sb": w1sb.astype(BF),
            "W2pad": w2pad.astype(BF),
            "W3pad": w3pad.astype(BF),
            "bvecs": bvecs.astype(BF),
            "uT": drecip[None, lo:lo + NODES_PER_CORE].astype(BF),
            "iota": iota_rep,
            **{f"idx{ch}": idx16[ch][c] for ch in range(NCHUNK)},
            "didx": didx[c].astype(BF),
        })
    return in_maps


_TRACE = [False]          # set by test harness to request a profiled run
_LAST_RESULT = [None]     # BassKernelResults of the last run (for profiling)


def kernel(x, edge_index, batch, W1, b1, W2, b2, W3, b3, Wfc, bfc):
    x = np.asarray(x)
    edge_index = np.asarray(edge_index)
    batch = np.asarray(batch)
    W1, b1 = np.asarray(W1), np.asarray(b1)
    W2, b2 = np.asarray(W2), np.asarray(b2)
    W3, b3 = np.asarray(W3), np.asarray(b3)
    Wfc, bfc = np.asarray(Wfc), np.asarray(bfc)

    dinv, drecip, idx16, didx, meta = _host_prep(edge_index.astype(np.int64))
    nc = _build_program(meta)
    in_maps = _pack_inputs(x, dinv, drecip, W1, b1, W2, b2, W3, b3, idx16, didx)
    res = run_bass_kernel_spmd(
        nc, in_maps, core_ids=list(range(N_CORES)), trace=_TRACE[0]
    )
    _LAST_RESULT[0] = res

    h3 = np.concatenate([res.results[c]["out3"][:, :32] for c in range(N_CORES)], axis=0)

    # host epilogue: segment max pool + FC + log_softmax (float64 for stability)
    pooled = np.full((N_GRAPHS, 32), -np.inf, np.float64)
    bnd = np.searchsorted(batch, np.arange(N_GRAPHS + 1))
    for g in range(N_GRAPHS):
        if bnd[g + 1] > bnd[g]:
            pooled[g] = h3[bnd[g]:bnd[g + 1]].max(axis=0)
    logits = pooled @ Wfc.astype(np.float64) + bfc.astype(np.float64)
    m = logits.max(axis=1, keepdims=True)
    lse = m + np.log(np.exp(logits - m).sum(axis=1, keepdims=True))
    return (logits - lse).astype(np.float32)


# revision 59
# speedup vs baseline: 1.2491x; 1.0865x over previous
"""GCN (3x GCNConv + global max pool + FC + log_softmax) on 8 Trainium2 NeuronCores.

Strategy:
  - 1D partition of nodes: core c owns rows [12500c, 12500(c+1)).
  - Per conv layer l: each core computes hs = dinv * (a @ W_l) for its slice
    (PE matmul, bf16). The per-core slice is split into 4 sub-chunks at tile
    boundaries (25/25/25/23 tiles); each sub-chunk is AllGather'd into a
    pair-shared DRAM table as soon as its dense tiles finish, so collectives
    overlap the remaining dense work and the next layer's gathers.
  - Aggregation: per-edge gather of table rows via gpsimd dma_gather (int16
    indices within each 25600-row chunk), then segment-sum via PE matmuls
    with per-128-edge one-hot selection matrices built on DVE (is_equal of
    dst-in-tile index vs an iota row), accumulated in PSUM per 128-dst tile.
    The bias enters as a rank-1 matmul (u=sqrt(deg) outer b) seeding PSUM;
    out = relu(dinv * psum) is a single scalar-engine activation.
  - dense(l+1) for tile t is fused right after agg(l) tile t (PE transpose
    feeds the next matmul), so layers pipeline without phase barriers.
  - Pooling (segment max over graphs), tiny FC and log_softmax run on host
    from the returned conv3 node features (0.01% of FLOPs).
"""

import sys

sys.path.insert(0, "/opt/trn_rl_repo")

import numpy as np
import ml_dtypes

import concourse.bass as bass
import concourse.bacc as bacc
import concourse.tile as tile
from concourse import mybir
from concourse.bass_utils import run_bass_kernel_spmd
from concourse.masks import make_identity

P = 128
N_NODES = 100000
N_EDGES = 1600000
N_GRAPHS = 64
N_CORES = 8
NODES_PER_CORE = N_NODES // N_CORES          # 12500
NTILES = (NODES_PER_CORE + P - 1) // P       # 98 (last tile 84 rows)
LAST_ROWS = NODES_PER_CORE - (NTILES - 1) * P  # 84
NCHUNK = 4
SUBTILES = [27, 27, 28, 16]                  # tiles per sub-chunk: fewest padded gather
SUBOFF_T = [0, 27, 54, 82]                   # slots on the real graph AND a short tail
SUBROWS = [3456, 3456, 3584, 2004]           # sub-AllGather (the layer-start gate)
SUBOFF_R = [0, 3456, 6912, 10496]
CHUNK_ROWS = [r * N_CORES for r in SUBROWS]  # 25600 x3, 23200
WT = 8                                       # tiles per wave
F0 = 512
FW = 128                                     # table width (all convs padded to 128)
MAX_CALL_BLOCKS = 47                         # ~6K idxs/call; single_packet=False required >1024
NQUEUES = 4
DMA_SCRATCH = 16384
dt = mybir.dt
BF = ml_dtypes.bfloat16


def _rows(t):
    return LAST_ROWS if t == NTILES - 1 else P


def _host_prep(edge_index):
    """Build the shared (cross-core) aggregation schedule + per-core index data."""
    src = np.concatenate([edge_index[0], np.arange(N_NODES, dtype=np.int64)]).astype(np.int64)
    dst = np.concatenate([edge_index[1], np.arange(N_NODES, dtype=np.int64)]).astype(np.int64)
    deg = np.bincount(dst, minlength=N_NODES).astype(np.float32)
    dinv = (1.0 / np.sqrt(deg)).astype(np.float32)
    drecip = np.sqrt(deg).astype(np.float32)  # 1/dinv, for the rank-1 bias matmul

    waves = [list(range(w, min(w + WT, NTILES))) for w in range(0, NTILES, WT)]

    # chunk id + index-in-chunk for every source node (sub-chunk row split)
    s_core = src // NODES_PER_CORE
    s_rl = src % NODES_PER_CORE
    ch_of = np.digitize(s_rl, SUBOFF_R[1:])
    suboff = np.asarray(SUBOFF_R, np.int64)[ch_of]
    subrows = np.asarray(SUBROWS, np.int64)[ch_of]
    s_inchunk = s_core * subrows + (s_rl - suboff)

    core_of = dst // NODES_PER_CORE
    per_core = []
    cnts = np.zeros((N_CORES, NTILES, NCHUNK), np.int64)
    for c in range(N_CORES):
        m = core_of == c
        s, d = s_inchunk[m], dst[m]
        chm = ch_of[m]
        dl = d - c * NODES_PER_CORE
        t = dl // P
        key = t * NCHUNK + chm
        o = np.argsort(key, kind="stable")
        s, dl, key = s[o], dl[o], key[o]
        cnt = np.bincount(key, minlength=NTILES * NCHUNK).reshape(NTILES, NCHUNK)
        cnts[c] = cnt
        per_core.append((s, dl, cnt))

    blocks = np.maximum((cnts.max(axis=0) + P - 1) // P, 1)  # [NTILES, NCHUNK] shared

    S_ch = [int(blocks[:, ch].sum() * P) for ch in range(NCHUNK)]
    NB_total = int(blocks.sum())

    idx16 = [np.zeros((N_CORES, 128, S_ch[ch] // 16), np.int16) for ch in range(NCHUNK)]
    didx = np.full((N_CORES, 128, NB_total), -1.0, np.float32)

    chunk_start = [dict() for _ in range(NCHUNK)]
    for ch in range(NCHUNK):
        pos = 0
        for w, wtiles in enumerate(waves):
            for t in wtiles:
                chunk_start[ch][t] = pos
                pos += int(blocks[t, ch]) * P
    g_start = {}
    g = 0
    for w, wtiles in enumerate(waves):
        for t in wtiles:
            for ch in range(NCHUNK):
                g_start[(t, ch)] = g
                g += int(blocks[t, ch])
    assert g == NB_total

    for c in range(N_CORES):
        s, dl, cnt = per_core[c]
        ends = np.cumsum(cnt.reshape(-1))
        starts = ends - cnt.reshape(-1)
        idx_slots = [np.zeros(S_ch[ch], np.int16) for ch in range(NCHUNK)]
        didx_slots = np.full(NB_total * P, -1.0, np.float32)
        for t in range(NTILES):
            for ch in range(NCHUNK):
                k = t * NCHUNK + ch
                n = cnt[t, ch]
                if n == 0:
                    continue
                e0, e1 = starts[k], ends[k]
                ss = s[e0:e1]
                dd = dl[e0:e1] - t * P
                o2 = np.argsort(ss, kind="stable")  # ascending srcs: HBM locality
                ss, dd = ss[o2], dd[o2]
                cs = chunk_start[ch][t]
                idx_slots[ch][cs:cs + n] = ss.astype(np.int16)
                gs = g_start[(t, ch)] * P
                didx_slots[gs:gs + n] = dd.astype(np.float32)
        for ch in range(NCHUNK):
            w16 = idx_slots[ch].reshape(-1, 16).T  # [16, S/16]
            idx16[ch][c] = np.tile(w16, (8, 1))
        didx[c] = didx_slots.reshape(-1, P).T

    meta = {
        "waves": waves,
        "blocks": blocks,
        "S_ch": S_ch,
        "NB_total": NB_total,
        "chunk_start": chunk_start,
        "g_start": g_start,
    }
    return dinv, drecip, idx16, didx, meta


def _build_program(meta):
    waves = meta["waves"]
    blocks = meta["blocks"]
    S_ch = meta["S_ch"]
    NB_total = meta["NB_total"]
    chunk_start = meta["chunk_start"]
    g_start = meta["g_start"]

    nc = bacc.Bacc(
        "TRN2", target_bir_lowering=False, debug=False, num_devices=N_CORES,
        num_swdge_queues=NQUEUES, dynamic_dma_scratch_size=DMA_SCRATCH,
    )

    xT_io = nc.dram_tensor("xT", [F0, NODES_PER_CORE], dt.bfloat16, kind="ExternalInput").ap()
    dinv_io = nc.dram_tensor("dinvT", [P, NTILES], dt.float32, kind="ExternalInput").ap()
    w1_io = nc.dram_tensor("W1sb", [P, F0], dt.bfloat16, kind="ExternalInput").ap()
    w2_io = nc.dram_tensor("W2pad", [P, P], dt.bfloat16, kind="ExternalInput").ap()
    w3_io = nc.dram_tensor("W3pad", [P, P], dt.bfloat16, kind="ExternalInput").ap()
    b_io = nc.dram_tensor("bvecs", [4, P], dt.bfloat16, kind="ExternalInput").ap()
    uT_io = nc.dram_tensor("uT", [1, NODES_PER_CORE], dt.bfloat16, kind="ExternalInput").ap()
    iota_io = nc.dram_tensor("iota", [P, P], dt.bfloat16, kind="ExternalInput").ap()
    idx_ios = [
        nc.dram_tensor(f"idx{ch}", [P, S_ch[ch] // 16], dt.int16, kind="ExternalInput").ap()
        for ch in range(NCHUNK)
    ]
    didx_io = nc.dram_tensor("didx", [P, NB_total], dt.bfloat16, kind="ExternalInput").ap()
    out_io = nc.dram_tensor("out3", [NODES_PER_CORE, 32], dt.float32, kind="ExternalOutput").ap()

    with tile.TileContext(nc) as tc:
        with (
            tc.tile_pool(name="const", bufs=1) as constp,
            tc.tile_pool(name="idxw", bufs=10) as idxp,
            tc.tile_pool(name="msgs", bufs=9) as msgp,
            tc.tile_pool(name="uTw", bufs=2) as uTp,
            tc.tile_pool(name="sel", bufs=14) as selp,
            tc.tile_pool(name="work", bufs=3) as workp,
            tc.tile_pool(name="hs", bufs=3) as hsp,
            tc.tile_pool(name="aT", bufs=3) as aTp,
            tc.tile_pool(name="psum_d", bufs=2, space="PSUM") as psdp,
            tc.tile_pool(name="psum_a", bufs=4, space="PSUM") as psap,
            tc.tile_pool(name="psum_t", bufs=2, space="PSUM") as pstp,
            tc.tile_pool(name="dram", bufs=1, space="DRAM") as dramp,
        ):
            # ---- constants ----
            iota_t = constp.tile([P, P], dt.bfloat16)
            nc.sync.dma_start(iota_t[:], iota_io[:])
            dinv_sb = constp.tile([P, NTILES], dt.float32)
            nc.sync.dma_start(dinv_sb[:], dinv_io[:])
            w1_sb = constp.tile([P, F0], dt.bfloat16)
            nc.sync.dma_start(w1_sb[:], w1_io[:])
            w2_sb = constp.tile([P, P], dt.bfloat16)
            nc.sync.dma_start(w2_sb[:], w2_io[:])
            w3_sb = constp.tile([P, P], dt.bfloat16)
            nc.sync.dma_start(w3_sb[:], w3_io[:])
            b_sbs = []
            for l in range(3):
                b_l = constp.tile([1, P], dt.bfloat16, name=f"b_sb{l}")
                nc.sync.dma_start(b_l[:], b_io[l:l + 1, :])
                b_sbs.append(b_l)
            didx_sb = constp.tile([P, NB_total], dt.bfloat16)
            nc.sync.dma_start(didx_sb[:], didx_io[:])
            ident = constp.tile([P, P], dt.float32)
            make_identity(nc, ident[:])

            import os as _os
            _shared = "Shared" if _os.environ.get("GCN_SHARED_TBL", "1") == "1" else "Local"
            tbls = [
                [
                    dramp.tile([CHUNK_ROWS[k], FW], dt.bfloat16,
                               tag=f"tbl{l}_{k}", name=f"tbl{l}_{k}", addr_space=_shared)
                    for k in range(NCHUNK)
                ]
                for l in range(3)
            ]
            bounces = [
                dramp.tile([NODES_PER_CORE, FW], dt.bfloat16, tag=f"bnc{l}", name=f"bnc{l}")
                for l in range(3)
            ]

            def sub_allgather(l, k):
                nc.gpsimd.collective_compute(
                    "AllGather", mybir.AluOpType.bypass,
                    replica_groups=[list(range(N_CORES))],
                    ins=[bounces[l][SUBOFF_R[k]:SUBOFF_R[k] + SUBROWS[k], :].opt()],
                    outs=[tbls[l][k].opt()],
                )

            boundary_of = {SUBOFF_T[k] + SUBTILES[k] - 1: k for k in range(NCHUNK)}

            def dense_tile(l, t, lhsT_full, w_sb, col0=0):
                """hs_t = dinv * (a_t @ W_l) -> bounce[l]."""
                r = _rows(t)
                c0 = t * P
                ps = psdp.tile([r, P], dt.float32, space="PSUM", tag="pd")
                if l == 0:
                    nk = F0 // P
                    for k in range(nk):
                        nc.tensor.matmul(
                            out=ps[:], lhsT=lhsT_full[:, k, col0:col0 + r],
                            rhs=w_sb[:, k * P:(k + 1) * P],
                            start=(k == 0), stop=(k == nk - 1),
                        )
                else:
                    nc.tensor.matmul(
                        out=ps[:], lhsT=lhsT_full[:, :r], rhs=w_sb[:],
                        start=True, stop=True,
                    )
                hs = hsp.tile([r, P], dt.bfloat16, tag="hs")
                nc.scalar.activation(
                    hs[:], ps[:], mybir.ActivationFunctionType.Copy,
                    scale=dinv_sb[:r, t:t + 1],
                )
                # scalar-engine DGE: keeps the sync queue free for idx loads
                nc.scalar.dma_start(bounces[l][c0:c0 + r, :], hs[:])
                if t in boundary_of:
                    sub_allgather(l, boundary_of[t])

            # ---- layer-0 dense: wave-sized bulk xT loads, then matmul per tile ----
            xT_r = xT_io.rearrange("(k p) n -> p k n", p=P)
            with tc.tile_pool(name="xT", bufs=2) as xTp:
                for w, wtiles in enumerate(waves):
                    c0w = wtiles[0] * P
                    cols = sum(_rows(t) for t in wtiles)
                    xw = xTp.tile([P, F0 // P, WT * P], dt.bfloat16, tag="xw")
                    nc.sync.dma_start(xw[:, :, :cols], xT_r[:, :, c0w:c0w + cols])
                    for ti, t in enumerate(wtiles):
                        dense_tile(0, t, xw, w1_sb, col0=ti * P)

            def agg_phase(l):
                """agg from tbls[l]; out = relu(dinv*(agg + u x b)); fused dense(l+1)."""
                ncols = P if l < 2 else 32

                def gather_one(w, ch):
                    wtiles = waves[w]
                    nb = int(sum(blocks[t, ch] for t in wtiles))
                    s0 = chunk_start[ch][wtiles[0]]
                    S = nb * P
                    iw = idxp.tile([P, S // 16], dt.int16, tag="idx")
                    nc.sync.dma_start(iw[:], idx_ios[ch][:, s0 // 16:(s0 + S) // 16])
                    mt = msgp.tile([P, nb, FW], dt.bfloat16, tag="msg")
                    q = ch % NQUEUES
                    for b0 in range(0, nb, MAX_CALL_BLOCKS):
                        b1 = min(b0 + MAX_CALL_BLOCKS, nb)
                        Ssub = (b1 - b0) * P
                        nc.gpsimd.dma_gather(
                            out_ap=mt[:, b0:b1, :],
                            in_ap=tbls[l][ch][:],
                            idxs_ap=iw[:, b0 * P // 16:b1 * P // 16],
                            num_idxs=Ssub, num_idxs_reg=Ssub,
                            elem_size=FW, elem_step=FW,
                            single_packet=False,
                            queue_num=q,
                        )
                    return mt

                for w, wtiles in enumerate(waves):
                    msg_tiles = {ch: gather_one(w, ch) for ch in range(NCHUNK)}
                    # per-wave u slice on the scalar DGE: keeps the sync queue
                    # (gather idx loads) clean and frees 25KB/part of const SBUF
                    uTw = uTp.tile([1, WT * P], dt.bfloat16, tag="uTw")
                    c0w = wtiles[0] * P
                    colsw = sum(_rows(t) for t in wtiles)
                    nc.scalar.dma_start(uTw[0:1, :colsw], uT_io[0:1, c0w:c0w + colsw])

                    # selection matrices, batches of BB blocks in didx col order
                    gw0 = g_start[(wtiles[0], 0)]
                    gw1 = gw0 + int(sum(blocks[t, ch] for t in wtiles for ch in range(NCHUNK)))
                    BB = 16
                    sel_tiles = {}
                    for q0 in range(gw0, gw1, BB):
                        q1 = min(q0 + BB, gw1)
                        st = selp.tile([P, BB, P], dt.bfloat16, tag="sel")
                        nc.vector.tensor_tensor(
                            out=st[:, :q1 - q0, :],
                            in0=didx_sb[:, q0:q1, None].to_broadcast([P, q1 - q0, P]),
                            in1=iota_t[:, None, :].to_broadcast([P, q1 - q0, P]),
                            op=mybir.AluOpType.is_equal,
                        )
                        for q in range(q0, q1):
                            sel_tiles[q] = (st, q - q0)

                    # per-tile matmul accumulation + post + fused next dense
                    for ti, t in enumerate(wtiles):
                        r = _rows(t)
                        c0 = t * P
                        ps = psap.tile([r, ncols], dt.float32, space="PSUM", tag="pa")
                        # rank-1 bias seed: ps = u[dst] (x) b, so that
                        # dinv * (agg + u (x) b) = dinv*agg + b
                        nc.tensor.matmul(
                            out=ps[:], lhsT=uTw[0:1, ti * P:ti * P + r],
                            rhs=b_sbs[l][0:1, :ncols],
                            start=True, stop=False,
                        )
                        mms = []
                        for ch in range(NCHUNK):
                            coff = int(sum(blocks[tt, ch] for tt in wtiles[:ti]))
                            for b in range(int(blocks[t, ch])):
                                mms.append((g_start[(t, ch)] + b, ch, coff + b))
                        for i, (q, ch, col) in enumerate(mms):
                            st, j = sel_tiles[q]
                            nc.tensor.matmul(
                                out=ps[:], lhsT=st[:, j, :r],
                                rhs=msg_tiles[ch][:, col, :ncols],
                                start=False, stop=(i == len(mms) - 1),
                            )
                        outt = workp.tile([P, ncols], dt.float32, tag="outt")
                        nc.scalar.activation(
                            outt[:r], ps[:], mybir.ActivationFunctionType.Relu,
                            scale=dinv_sb[:r, t:t + 1],
                        )
                        if l < 2:
                            pst = pstp.tile([P, P], dt.float32, space="PSUM", tag="pt")
                            nc.tensor.transpose(out=pst[:, :r], in_=outt[:r], identity=ident[:r, :r])
                            aTt = aTp.tile([P, P], dt.bfloat16, tag="aTt")
                            # scalar engine: keep DVE free for sel builds
                            nc.scalar.activation(
                                aTt[:, :r], pst[:, :r],
                                mybir.ActivationFunctionType.Copy,
                            )
                            dense_tile(l + 1, t, aTt, w2_sb if l == 0 else w3_sb)
                        else:
                            nc.scalar.dma_start(out_io[c0:c0 + r, :], outt[:r])

            for l in range(3):
                agg_phase(l)

    nc.compile()
    return nc


def _pack_inputs(x, dinv, drecip, W1, b1, W2, b2, W3, b3, idx16, didx):
    iota_rep = np.tile(np.arange(P, dtype=np.float32)[None, :], (P, 1)).astype(BF)
    # W1 packed: W1sb[i, 128k+j] = W1[128k+i, j]
    w1sb = np.zeros((P, F0), np.float32)
    for k in range(F0 // P):
        w1sb[:, k * P:(k + 1) * P] = W1[k * P:(k + 1) * P, :]
    w2pad = np.zeros((P, P), np.float32)
    w2pad[:, :64] = W2
    w3pad = np.zeros((P, P), np.float32)
    w3pad[:64, :32] = W3
    bvecs = np.zeros((4, P), np.float32)
    bvecs[0, :128] = b1
    bvecs[1, :64] = b2
    bvecs[2, :32] = b3

    in_maps = []
    for c in range(N_CORES):
        lo = c * NODES_PER_CORE
        xs = x[lo:lo + NODES_PER_CORE].astype(np.float32)
        dvt = np.ones((P, NTILES), np.float32)
        dv = dinv[lo:lo + NODES_PER_CORE]
        for t in range(NTILES):
            r = _rows(t)
            dvt[:r, t] = dv[t * P:t * P + r]
        in_maps.append({
            "xT": np.ascontiguousarray(xs.T).astype(BF),
            "dinvT": dvt,
            "W1sb": w1sb.astype(BF),
            "W2pad": w2pad.astype(BF),
            "W3pad": w3pad.astype(BF),
            "bvecs": bvecs.astype(BF),
            "uT": drecip[None, lo:lo + NODES_PER_CORE].astype(BF),
            "iota": iota_rep,
            **{f"idx{ch}": idx16[ch][c] for ch in range(NCHUNK)},
            "didx": didx[c].astype(BF),
        })
    return in_maps


_TRACE = [False]          # set by test harness to request a profiled run
_LAST_RESULT = [None]     # BassKernelResults of the last run (for profiling)


def kernel(x, edge_index, batch, W1, b1, W2, b2, W3, b3, Wfc, bfc):
    x = np.asarray(x)
    edge_index = np.asarray(edge_index)
    batch = np.asarray(batch)
    W1, b1 = np.asarray(W1), np.asarray(b1)
    W2, b2 = np.asarray(W2), np.asarray(b2)
    W3, b3 = np.asarray(W3), np.asarray(b3)
    Wfc, bfc = np.asarray(Wfc), np.asarray(bfc)

    dinv, drecip, idx16, didx, meta = _host_prep(edge_index.astype(np.int64))
    nc = _build_program(meta)
    in_maps = _pack_inputs(x, dinv, drecip, W1, b1, W2, b2, W3, b3, idx16, didx)
    res = run_bass_kernel_spmd(
        nc, in_maps, core_ids=list(range(N_CORES)), trace=_TRACE[0]
    )
    _LAST_RESULT[0] = res

    h3 = np.concatenate([res.results[c]["out3"][:, :32] for c in range(N_CORES)], axis=0)

    # host epilogue: segment max pool + FC + log_softmax (float64 for stability)
    pooled = np.full((N_GRAPHS, 32), -np.inf, np.float64)
    bnd = np.searchsorted(batch, np.arange(N_GRAPHS + 1))
    for g in range(N_GRAPHS):
        if bnd[g + 1] > bnd[g]:
            pooled[g] = h3[bnd[g]:bnd[g + 1]].max(axis=0)
    logits = pooled @ Wfc.astype(np.float64) + bfc.astype(np.float64)
    m = logits.max(axis=1, keepdims=True)
    lse = m + np.log(np.exp(logits - m).sum(axis=1, keepdims=True))
    return (logits - lse).astype(np.float32)
